# revision 46
# baseline (speedup 1.0000x reference)
"""Trainium2 Bass kernel for the CoAttention scoring layer.

reference:
    keys    = receiver @ w_k                      # [B, R, D]
    queries = attendant @ w_q                     # [B, A, D]
    e_act   = queries[:, None, :, :] + keys[:, :, None, :] + bias  # [B, R, A, D]
    out     = tanh(e_act) @ a                     # [B, R, A]

Sharding: data-parallel over B across 8 NeuronCores (8 batches per core),
params replicated.

Host prep: inputs are pre-transposed to (F, R)/(F, A) layout and cast to
fp16 (device DMA cannot cast, and fp16 matmuls are single-pass on the PE
while fp32 lowers to LOW/HIGH double passes).

Per-core device layout (D=128 in the partition dim):
    kT (D, R), qT (D, A), k_nat (R, D), q_nat (A, D)  via fp16 matmuls
    e chunks (D, CA*R): e[d, j, r] = qbT[d, a0+j] + kT[d, r]
        - most chunks: one DVE broadcast tensor_add (SBUF)
        - some chunks: PE identity-selector matmuls into PSUM
    th = tanh(e) on ACT (fp16 out; PSUM-sourced chunks fold bias as the
        per-partition activation bias)
    scores[:, a] = th_a^T @ a_vec: per-a fp16 matvec, (R, A) PSUM tile
    scores -> SBUF -> DMA per batch.
"""

import sys

if "/opt/trn_rl_repo" not in sys.path:
    sys.path.insert(0, "/opt/trn_rl_repo")

from contextlib import ExitStack

import numpy as np

import concourse.bacc as bacc
import concourse.tile as tile
from concourse import mybir
from concourse.bass_utils import run_bass_kernel_spmd

B, R, A, F = 64, 128, 128, 256
D = F // 2
NCORES = 8
BC = B // NCORES  # batches per core
CA = 32           # a-indices per chunk (free dim = CA * R = 4096)
NCHUNK = A // CA  # chunks per batch
PE_CHUNKS = (3,)  # chunk indices whose broadcast-add runs on the PE
EP_FD = 512       # PSUM e-tile free dim (one bank; fp32 matmul-out limit)
JE = EP_FD // R   # a-indices per PSUM e-tile
F32 = mybir.dt.float32
F16 = mybir.dt.float16

_CACHE = {}


def build_bass():
    nc = bacc.Bacc("TRN2", target_bir_lowering=False, debug=False)

    rat_d = nc.declare_dram_parameter("rat16", [BC, 128, 4, 128], F16, isOutput=False)
    wqk_d = nc.declare_dram_parameter("wqk16", [128, 4, 128], F16, isOutput=False)
    bias_d = nc.declare_dram_parameter("bias", [D, 1], F32, isOutput=False)
    a_d = nc.declare_dram_parameter("a16", [D, 1], F16, isOutput=False)
    selq_d = nc.declare_dram_parameter("selq16", [A, len(PE_CHUNKS) * CA * R], F16, isOutput=False)
    selr_d = nc.declare_dram_parameter("selr16", [R, JE * R], F16, isOutput=False)
    out = nc.declare_dram_parameter("out", [BC, R, A], F32, isOutput=True)

    TANH = mybir.ActivationFunctionType.Tanh

    with tile.TileContext(nc) as tc, ExitStack() as ctx:
        const = ctx.enter_context(tc.tile_pool(name="const", bufs=1))
        natp = ctx.enter_context(tc.tile_pool(name="nat", bufs=3))
        kqp = ctx.enter_context(tc.tile_pool(name="kqp", bufs=2, space="PSUM"))
        kqs = ctx.enter_context(tc.tile_pool(name="kqs", bufs=3))
        ep = ctx.enter_context(tc.tile_pool(name="ep", bufs=4))
        epp = ctx.enter_context(tc.tile_pool(name="epp", bufs=2, space="PSUM"))
        thp = ctx.enter_context(tc.tile_pool(name="thp", bufs=4))
        scp = ctx.enter_context(tc.tile_pool(name="scp", bufs=2, space="PSUM"))
        scs = ctx.enter_context(tc.tile_pool(name="scs", bufs=2))

        # DMA issue order matters: the Sync engine serializes HWDGE issues at
        # ~0.6-0.9us each, so batch 0's input and the weights go first.
        rat_tiles = {}
        rat0 = natp.tile([128, 4, 128], F16, tag="rat", name="rat0")
        rat_tiles[0] = rat0
        nc.sync.dma_start(rat_tiles[0][:], rat_d[0])
        # packed weights: rows 0..255 = w_k, 256..511 = w_q, loaded in one DMA
        wqk_sb = const.tile([128, 4, 128], F16, tag="wqk")
        nc.sync.dma_start(wqk_sb[:], wqk_d[:])
        wk_sb = wqk_sb[:].rearrange("p g r -> p (g r)")[:, 0:256]
        wq_sb = wqk_sb[:].rearrange("p g r -> p (g r)")[:, 256:512]
        bias_col = const.tile([D, 1], F32, tag="bias")
        nc.sync.dma_start(bias_col[:], bias_d[:])
        a_f16 = const.tile([D, 1], F16, tag="avec16")
        nc.sync.dma_start(a_f16[:], a_d[:])
        selq_sb = const.tile([A, len(PE_CHUNKS) * CA * R], F16, tag="selq")
        nc.sync.dma_start(selq_sb[:], selq_d[:])
        selr_sb = const.tile([R, JE * R], F16, tag="selr")
        nc.sync.dma_start(selr_sb[:], selr_d[:])

        pending_sc = None  # (sc_ps tile, batch): evicted during next batch's
        # setup so the DVE never stalls on the current batch's reduction MMs
        for b in range(BC):
            # packed transposed fp16 inputs, one DMA per batch:
            # free blocks 0,1 = receiverT f-tiles; 2,3 = attendantT f-tiles
            if b in rat_tiles:
                rat = rat_tiles[b]
            else:
                rat = natp.tile([128, 4, 128], F16, tag="rat")
                nc.sync.dma_start(rat[:], rat_d[b])
            flat = rat[:].rearrange("p g r -> p (g r)")
            rT = flat[:, 0:256]
            aT = flat[:, 256:512]

            # kT = w_k^T @ recv^T (D, R); k_nat = recv @ w_k (R, D); same for q
            kT_ps = kqp.tile([D, R], F32, tag="kq")
            nc.tensor.matmul(kT_ps[:], wk_sb[:, 0:D], rT[:, 0:128], start=True, stop=False)
            nc.tensor.matmul(kT_ps[:], wk_sb[:, D:2 * D], rT[:, 128:256], start=False, stop=True)
            kT_sb = kqs.tile([D, R], F32, tag="kT_sb")
            nc.vector.tensor_copy(kT_sb[:], kT_ps[:])

            qT_ps = kqp.tile([D, A], F32, tag="kq")
            nc.tensor.matmul(qT_ps[:], wq_sb[:, 0:D], aT[:, 0:128], start=True, stop=False)
            nc.tensor.matmul(qT_ps[:], wq_sb[:, D:2 * D], aT[:, 128:256], start=False, stop=True)
            qbT_sb = kqs.tile([D, A], F32, tag="qbT_sb")
            nc.vector.tensor_scalar_add(qbT_sb[:], qT_ps[:], bias_col[:, 0:1])

            kn_ps = kqp.tile([R, D], F32, tag="kq")
            nc.tensor.matmul(kn_ps[:], rT[:, 0:128], wk_sb[:, 0:D], start=True, stop=False)
            nc.tensor.matmul(kn_ps[:], rT[:, 128:256], wk_sb[:, D:2 * D], start=False, stop=True)
            k_nat = kqs.tile([R, D], F16, tag="k_nat")
            nc.vector.tensor_copy(k_nat[:], kn_ps[:])

            qn_ps = kqp.tile([A, D], F32, tag="kq")
            nc.tensor.matmul(qn_ps[:], aT[:, 0:128], wq_sb[:, 0:D], start=True, stop=False)
            nc.tensor.matmul(qn_ps[:], aT[:, 128:256], wq_sb[:, D:2 * D], start=False, stop=True)
            q_nat = kqs.tile([A, D], F16, tag="q_nat")
            nc.vector.tensor_copy(q_nat[:], qn_ps[:])

            if pending_sc is not None:
                sb_prev, b_prev = pending_sc
                sc_sb = scs.tile([R, A], F32, tag="sc_sb")
                nc.vector.tensor_copy(sc_sb[:], sb_prev[:])
                nc.sync.dma_start(out[b_prev], sc_sb[:])

            sc_ps = scp.tile([R, A], F32, tag="sc_ps")
            # Interleave: PE chunk early, so its PSUM-sourced ACT reads fill
            # the ACT idle slivers while DVE computes the later chunks; for
            # the last batch it goes first so the pipeline drains via DVE.
            dve_chunks = [c for c in range(NCHUNK) if c not in PE_CHUNKS]
            pec = PE_CHUNKS[0]
            if b == BC - 1:
                items = [(pec, (0, 4))] + [(c, None) for c in dve_chunks]
            else:
                # split the PE chunk's PSUM tiles across both ACT slivers
                items = [
                    (dve_chunks[0], None),
                    (pec, (0, 2)),
                    (dve_chunks[1], None),
                    (pec, (2, 4)),
                    (dve_chunks[2], None),
                ]
            th_pe = None
            EPT = 1024           # psum e-tile width (2 banks)
            NSL = EPT // EP_FD   # matmul slices per psum tile
            j8 = EPT // R        # a-indices per psum tile
            for ac, srange in items:
                a0 = ac * CA
                if ac in PE_CHUNKS:
                    # PE path: e[d, (j, r)] = q_nat^T selectA + k_nat^T selectR
                    # in 512-col PSUM slices; ACT folds +bias during tanh.
                    if th_pe is None:
                        th_pe = thp.tile([D, CA, R], F16, tag="th")
                    th = th_pe
                    for s2 in range(*srange):
                        e_ps = epp.tile([D, EPT], F32, tag="e_ps")
                        for t in range(NSL):
                            o = (s2 * NSL + t) * EP_FD
                            nc.tensor.matmul(
                                e_ps[:, t * EP_FD:(t + 1) * EP_FD],
                                q_nat[:],
                                selq_sb[:, o:o + EP_FD],
                                start=True,
                                stop=False,
                            )
                            nc.tensor.matmul(
                                e_ps[:, t * EP_FD:(t + 1) * EP_FD],
                                k_nat[:],
                                selr_sb[:],
                                start=False,
                                stop=True,
                            )
                        nc.scalar.activation(
                            th[:, s2 * j8:(s2 + 1) * j8], e_ps[:], TANH, bias=bias_col[:, 0:1]
                        )
                    jlo, jhi = srange[0] * j8, srange[1] * j8
                else:
                    # DVE path: broadcast add(s), then tanh; ramp first/last
                    th = thp.tile([D, CA, R], F16, tag="th")
                    e = ep.tile([D, CA, R], F32, tag="e")
                    ramp = (b == 0 and ac == dve_chunks[0]) or b == BC - 1
                    CS = CA // 4 if ramp else CA
                    for c0 in range(0, CA, CS):
                        in0 = (
                            qbT_sb[:, a0 + c0:a0 + c0 + CS]
                            .unsqueeze(2)
                            .broadcast_to([D, CS, R])
                        )
                        in1 = kT_sb[:].unsqueeze(1).broadcast_to([D, CS, R])
                        nc.vector.tensor_add(e[:, c0:c0 + CS], in0, in1)
                        nc.scalar.activation(th[:, c0:c0 + CS], e[:, c0:c0 + CS], TANH)
                    jlo, jhi = 0, CA
                for j in range(jlo, jhi):
                    nc.tensor.matmul(
                        sc_ps[:, a0 + j:a0 + j + 1],
                        th[:, j],
                        a_f16[:],
                        start=True,
                        stop=True,
                    )
            pending_sc = (sc_ps, b)
        sb_prev, b_prev = pending_sc
        sc_sb = scs.tile([R, A], F32, tag="sc_sb")
        nc.vector.tensor_copy(sc_sb[:], sb_prev[:])
        nc.sync.dma_start(out[b_prev], sc_sb[:])

    nc.finalize()
    return nc


def _get_nc():
    if "nc" not in _CACHE:
        _CACHE["nc"] = build_bass()
    return _CACHE["nc"]


def make_in_maps(inputs):
    receiver = np.ascontiguousarray(inputs["receiver"], dtype=np.float32)
    attendant = np.ascontiguousarray(inputs["attendant"], dtype=np.float32)
    w_q16 = np.asarray(inputs["w_q"], dtype=np.float16)
    w_k16 = np.asarray(inputs["w_k"], dtype=np.float16)
    wqk16 = np.ascontiguousarray(
        np.concatenate([w_k16, w_q16], axis=0)
        .reshape(4, 128, 128)
        .transpose(1, 0, 2)
    )
    bias = np.ascontiguousarray(inputs["bias"], dtype=np.float32).reshape(D, 1)
    a16 = np.ascontiguousarray(inputs["a"], dtype=np.float16).reshape(D, 1)
    recvT16 = receiver.transpose(0, 2, 1).astype(np.float16)
    attnT16 = attendant.transpose(0, 2, 1).astype(np.float16)
    rat16 = np.ascontiguousarray(
        np.concatenate([recvT16, attnT16], axis=1)
        .reshape(-1, 4, 128, 128)
        .transpose(0, 2, 1, 3)
    )
    # selector constants for the PE broadcast-add chunks
    eye = np.eye(A, dtype=np.float16)
    selq = np.zeros((A, len(PE_CHUNKS), CA, R), dtype=np.float16)
    for i, pc in enumerate(PE_CHUNKS):
        selq[:, i, :, :] = eye[:, pc * CA:(pc + 1) * CA][:, :, None]
    selq16 = np.ascontiguousarray(selq.reshape(A, -1))
    selr16 = np.ascontiguousarray(
        np.broadcast_to(eye[:, None, :], (R, JE, R)).reshape(R, -1).astype(np.float16)
    )
    in_maps = []
    for c in range(NCORES):
        in_maps.append(
            {
                "rat16": np.ascontiguousarray(rat16[c * BC:(c + 1) * BC]),
                "wqk16": wqk16,
                "bias": bias,
                "a16": a16,
                "selq16": selq16,
                "selr16": selr16,
            }
        )
    return in_maps


def run(inputs, **kwargs):
    nc = _get_nc()
    in_maps = make_in_maps(inputs)
    res = run_bass_kernel_spmd(nc, in_maps, list(range(NCORES)), **kwargs)
    out = np.concatenate([res.results[c]["out"] for c in range(NCORES)], axis=0)
    return out, res


def kernel(**inputs) -> np.ndarray:
    out, _ = run(inputs)
    return out


# revision 47
# speedup vs baseline: 1.0261x; 1.0261x over previous
"""Trainium2 Bass kernel for the CoAttention scoring layer.

reference:
    keys    = receiver @ w_k                      # [B, R, D]
    queries = attendant @ w_q                     # [B, A, D]
    e_act   = queries[:, None, :, :] + keys[:, :, None, :] + bias  # [B, R, A, D]
    out     = tanh(e_act) @ a                     # [B, R, A]

Sharding: data-parallel over B across 8 NeuronCores (8 batches per core),
params replicated.

Host prep: inputs are pre-transposed to (F, R)/(F, A) layout and cast to
fp16 (device DMA cannot cast, and fp16 matmuls are single-pass on the PE
while fp32 lowers to LOW/HIGH double passes).

Per-core device layout (D=128 in the partition dim):
    kT (D, R), qT (D, A), k_nat (R, D), q_nat (A, D)  via fp16 matmuls
    e chunks (D, CA*R): e[d, j, r] = qbT[d, a0+j] + kT[d, r]
        - most chunks: one DVE broadcast tensor_add (SBUF)
        - some chunks: PE identity-selector matmuls into PSUM
    th = tanh(e) on ACT (fp16 out; PSUM-sourced chunks fold bias as the
        per-partition activation bias)
    scores[:, a] = th_a^T @ a_vec: per-a fp16 matvec, (R, A) PSUM tile
    scores -> SBUF -> DMA per batch.
"""

import sys

if "/opt/trn_rl_repo" not in sys.path:
    sys.path.insert(0, "/opt/trn_rl_repo")

from contextlib import ExitStack

import numpy as np

import concourse.bacc as bacc
import concourse.tile as tile
from concourse import mybir
from concourse.bass_utils import run_bass_kernel_spmd

B, R, A, F = 64, 128, 128, 256
D = F // 2
NCORES = 8
BC = B // NCORES  # batches per core
CA = 32           # a-indices per chunk (free dim = CA * R = 4096)
NCHUNK = A // CA  # chunks per batch
PE_CHUNKS = (3,)  # chunk indices whose broadcast-add runs on the PE
EP_FD = 512       # PSUM e-tile free dim (one bank; fp32 matmul-out limit)
JE = EP_FD // R   # a-indices per PSUM e-tile
F32 = mybir.dt.float32
F16 = mybir.dt.float16

_CACHE = {}


def build_bass():
    nc = bacc.Bacc("TRN2", target_bir_lowering=False, debug=False)

    rat_d = nc.declare_dram_parameter("rat16", [BC, 128, 4, 128], F16, isOutput=False)
    wqk_d = nc.declare_dram_parameter("wqk16", [128, 4, 128], F16, isOutput=False)
    bias_d = nc.declare_dram_parameter("bias", [D, 1], F32, isOutput=False)
    a_d = nc.declare_dram_parameter("a16", [D, 1], F16, isOutput=False)
    selq_d = nc.declare_dram_parameter("selq16", [A, len(PE_CHUNKS) * CA * R], F16, isOutput=False)
    selr_d = nc.declare_dram_parameter("selr16", [R, JE * R], F16, isOutput=False)
    out = nc.declare_dram_parameter("out", [BC, R, A], F32, isOutput=True)

    TANH = mybir.ActivationFunctionType.Tanh

    with tile.TileContext(nc) as tc, ExitStack() as ctx:
        const = ctx.enter_context(tc.tile_pool(name="const", bufs=1))
        natp = ctx.enter_context(tc.tile_pool(name="nat", bufs=3))
        kqp = ctx.enter_context(tc.tile_pool(name="kqp", bufs=2, space="PSUM"))
        kqs = ctx.enter_context(tc.tile_pool(name="kqs", bufs=3))
        ep = ctx.enter_context(tc.tile_pool(name="ep", bufs=4))
        epp = ctx.enter_context(tc.tile_pool(name="epp", bufs=2, space="PSUM"))
        thp = ctx.enter_context(tc.tile_pool(name="thp", bufs=4))
        scp = ctx.enter_context(tc.tile_pool(name="scp", bufs=2, space="PSUM"))
        scs = ctx.enter_context(tc.tile_pool(name="scs", bufs=2))

        # DMA issue order matters: the Sync engine serializes HWDGE issues at
        # ~0.6-0.9us each, so batch 0's input and the weights go first.
        rat_tiles = {}
        rat0 = natp.tile([128, 4, 128], F16, tag="rat", name="rat0")
        rat_tiles[0] = rat0
        nc.sync.dma_start(rat_tiles[0][:], rat_d[0])
        # packed weights: rows 0..255 = w_k, 256..511 = w_q, loaded in one DMA
        wqk_sb = const.tile([128, 4, 128], F16, tag="wqk")
        nc.sync.dma_start(wqk_sb[:], wqk_d[:])
        wk_sb = wqk_sb[:].rearrange("p g r -> p (g r)")[:, 0:256]
        wq_sb = wqk_sb[:].rearrange("p g r -> p (g r)")[:, 256:512]
        bias_col = const.tile([D, 1], F32, tag="bias")
        nc.sync.dma_start(bias_col[:], bias_d[:])
        a_f16 = const.tile([D, 1], F16, tag="avec16")
        nc.sync.dma_start(a_f16[:], a_d[:])
        selq_sb = const.tile([A, len(PE_CHUNKS) * CA * R], F16, tag="selq")
        nc.sync.dma_start(selq_sb[:], selq_d[:])
        selr_sb = const.tile([R, JE * R], F16, tag="selr")
        nc.sync.dma_start(selr_sb[:], selr_d[:])

        pending_sc = None  # (sc_ps tile, batch): evicted during next batch's
        # setup so the DVE never stalls on the current batch's reduction MMs
        for b in range(BC):
            # packed transposed fp16 inputs, one DMA per batch:
            # free blocks 0,1 = receiverT f-tiles; 2,3 = attendantT f-tiles
            if b in rat_tiles:
                rat = rat_tiles[b]
            else:
                rat = natp.tile([128, 4, 128], F16, tag="rat")
                nc.sync.dma_start(rat[:], rat_d[b])
            flat = rat[:].rearrange("p g r -> p (g r)")
            rT = flat[:, 0:256]
            aT = flat[:, 256:512]

            # kT = w_k^T @ recv^T (D, R); k_nat = recv @ w_k (R, D); same for q
            kT_ps = kqp.tile([D, R], F32, tag="kq")
            nc.tensor.matmul(kT_ps[:], wk_sb[:, 0:D], rT[:, 0:128], start=True, stop=False)
            nc.tensor.matmul(kT_ps[:], wk_sb[:, D:2 * D], rT[:, 128:256], start=False, stop=True)
            kT_sb = kqs.tile([D, R], F32, tag="kT_sb")
            nc.vector.tensor_copy(kT_sb[:], kT_ps[:])

            qT_ps = kqp.tile([D, A], F32, tag="kq")
            nc.tensor.matmul(qT_ps[:], wq_sb[:, 0:D], aT[:, 0:128], start=True, stop=False)
            nc.tensor.matmul(qT_ps[:], wq_sb[:, D:2 * D], aT[:, 128:256], start=False, stop=True)
            qbT_sb = kqs.tile([D, A], F32, tag="qbT_sb")
            nc.vector.tensor_scalar_add(qbT_sb[:], qT_ps[:], bias_col[:, 0:1])

            kn_ps = kqp.tile([R, D], F32, tag="kq")
            nc.tensor.matmul(kn_ps[:], rT[:, 0:128], wk_sb[:, 0:D], start=True, stop=False)
            nc.tensor.matmul(kn_ps[:], rT[:, 128:256], wk_sb[:, D:2 * D], start=False, stop=True)
            k_nat = kqs.tile([R, D], F16, tag="k_nat")
            nc.vector.tensor_copy(k_nat[:], kn_ps[:])

            qn_ps = kqp.tile([A, D], F32, tag="kq")
            nc.tensor.matmul(qn_ps[:], aT[:, 0:128], wq_sb[:, 0:D], start=True, stop=False)
            nc.tensor.matmul(qn_ps[:], aT[:, 128:256], wq_sb[:, D:2 * D], start=False, stop=True)
            q_nat = kqs.tile([A, D], F16, tag="q_nat")
            nc.vector.tensor_copy(q_nat[:], qn_ps[:])

            if pending_sc is not None:
                sb_prev, b_prev = pending_sc
                sc_sb = scs.tile([R, A], F32, tag="sc_sb")
                nc.vector.tensor_copy(sc_sb[:], sb_prev[:])
                nc.sync.dma_start(out[b_prev], sc_sb[:])

            sc_ps = scp.tile([R, A], F32, tag="sc_ps")
            # Interleave: PE chunk early, so its PSUM-sourced ACT reads fill
            # the ACT idle slivers while DVE computes the later chunks; for
            # the last batch it goes first so the pipeline drains via DVE.
            dve_chunks = [c for c in range(NCHUNK) if c not in PE_CHUNKS]
            if b == BC - 1:
                order = list(PE_CHUNKS) + dve_chunks
            else:
                order = dve_chunks[:1] + list(PE_CHUNKS) + dve_chunks[1:]
            for ac in order:
                a0 = ac * CA
                th = thp.tile([D, CA, R], F16, tag="th")
                if ac in PE_CHUNKS:
                    # PE path: e[d, (j, r)] = q_nat^T selectA + k_nat^T selectR
                    # in 512-col PSUM slices; ACT folds +bias during tanh.
                    pci = PE_CHUNKS.index(ac)
                    EPT = 1024           # psum e-tile width (2 banks)
                    NSL = EPT // EP_FD   # matmul slices per psum tile
                    j8 = EPT // R        # a-indices per psum tile
                    for s in range(CA * R // EPT):
                        e_ps = epp.tile([D, EPT], F32, tag="e_ps")
                        for t in range(NSL):
                            o = (pci * CA // JE + s * NSL + t) * EP_FD
                            nc.tensor.matmul(
                                e_ps[:, t * EP_FD:(t + 1) * EP_FD],
                                q_nat[:],
                                selq_sb[:, o:o + EP_FD],
                                start=True,
                                stop=False,
                            )
                            nc.tensor.matmul(
                                e_ps[:, t * EP_FD:(t + 1) * EP_FD],
                                k_nat[:],
                                selr_sb[:],
                                start=False,
                                stop=True,
                            )
                        nc.scalar.activation(
                            th[:, s * j8:(s + 1) * j8], e_ps[:], TANH, bias=bias_col[:, 0:1]
                        )
                else:
                    # DVE path: broadcast add(s), then tanh. The very first and
                    # last chunks of the kernel run in quarter pieces so the
                    # ACT pipeline ramps up / drains with less idle time.
                    e = ep.tile([D, CA, R], F32, tag="e")
                    ramp = (b == 0 and ac == 0) or b == BC - 1
                    CS = CA // 4 if ramp else CA
                    for c0 in range(0, CA, CS):
                        in0 = (
                            qbT_sb[:, a0 + c0:a0 + c0 + CS]
                            .unsqueeze(2)
                            .broadcast_to([D, CS, R])
                        )
                        in1 = kT_sb[:].unsqueeze(1).broadcast_to([D, CS, R])
                        nc.vector.tensor_add(e[:, c0:c0 + CS], in0, in1)
                        nc.scalar.activation(th[:, c0:c0 + CS], e[:, c0:c0 + CS], TANH)
                for j in range(CA):
                    nc.tensor.matmul(
                        sc_ps[:, a0 + j:a0 + j + 1],
                        th[:, j],
                        a_f16[:],
                        start=True,
                        stop=True,
                    )
            pending_sc = (sc_ps, b)
        sb_prev, b_prev = pending_sc
        sc_sb = scs.tile([R, A], F32, tag="sc_sb")
        nc.vector.tensor_copy(sc_sb[:], sb_prev[:])
        nc.sync.dma_start(out[b_prev], sc_sb[:])

    nc.finalize()
    return nc


def _get_nc():
    if "nc" not in _CACHE:
        _CACHE["nc"] = build_bass()
    return _CACHE["nc"]


def make_in_maps(inputs):
    receiver = np.ascontiguousarray(inputs["receiver"], dtype=np.float32)
    attendant = np.ascontiguousarray(inputs["attendant"], dtype=np.float32)
    w_q16 = np.asarray(inputs["w_q"], dtype=np.float16)
    w_k16 = np.asarray(inputs["w_k"], dtype=np.float16)
    wqk16 = np.ascontiguousarray(
        np.concatenate([w_k16, w_q16], axis=0)
        .reshape(4, 128, 128)
        .transpose(1, 0, 2)
    )
    bias = np.ascontiguousarray(inputs["bias"], dtype=np.float32).reshape(D, 1)
    a16 = np.ascontiguousarray(inputs["a"], dtype=np.float16).reshape(D, 1)
    recvT16 = receiver.transpose(0, 2, 1).astype(np.float16)
    attnT16 = attendant.transpose(0, 2, 1).astype(np.float16)
    rat16 = np.ascontiguousarray(
        np.concatenate([recvT16, attnT16], axis=1)
        .reshape(-1, 4, 128, 128)
        .transpose(0, 2, 1, 3)
    )
    # selector constants for the PE broadcast-add chunks
    eye = np.eye(A, dtype=np.float16)
    selq = np.zeros((A, len(PE_CHUNKS), CA, R), dtype=np.float16)
    for i, pc in enumerate(PE_CHUNKS):
        selq[:, i, :, :] = eye[:, pc * CA:(pc + 1) * CA][:, :, None]
    selq16 = np.ascontiguousarray(selq.reshape(A, -1))
    selr16 = np.ascontiguousarray(
        np.broadcast_to(eye[:, None, :], (R, JE, R)).reshape(R, -1).astype(np.float16)
    )
    in_maps = []
    for c in range(NCORES):
        in_maps.append(
            {
                "rat16": np.ascontiguousarray(rat16[c * BC:(c + 1) * BC]),
                "wqk16": wqk16,
                "bias": bias,
                "a16": a16,
                "selq16": selq16,
                "selr16": selr16,
            }
        )
    return in_maps


def run(inputs, **kwargs):
    nc = _get_nc()
    in_maps = make_in_maps(inputs)
    res = run_bass_kernel_spmd(nc, in_maps, list(range(NCORES)), **kwargs)
    out = np.concatenate([res.results[c]["out"] for c in range(NCORES)], axis=0)
    return out, res


def kernel(**inputs) -> np.ndarray:
    out, _ = run(inputs)
    return out
